# revision 1
# baseline (speedup 1.0000x reference)
"""Causal self-attention (value-residual + QK RMSNorm + RoPE + sigmoid gate)
Trainium2 Bass kernel, sharded over 8 NeuronCores.

Sharding: core c handles batch b = c // 4 and the 4 heads [4*(c%4), 4*(c%4)+4).
Each core computes its heads' QKV, attention and gating, then a partial
c_proj contribution out_partial^T = Wproj[:, islice] @ y_gated^T  [1024, 2048].
Host sums the 4 partials per batch and transposes back.

All heavy matmuls run as float32r (full fp32 storage, TF32-ish PE path which
streams at bf16 rate for free dims >= 256). Softmax uses exp only; RMSNorm
rsqrt is computed as exp(-0.5*ln(.)) so the whole kernel needs one ACT table
set (natural_log_exp_and_others) — no table reloads.
"""

import sys

sys.path.insert(0, "/opt/trn_rl_repo")

import math

import numpy as np

import concourse.bass as bass
import concourse.mybir as mybir
import concourse.tile as tile
from concourse.bass_utils import run_bass_kernel_spmd
from concourse import bacc


# Force Exp and Ln to resolve to the combined natural_log_exp_and_others set:
# the greedy table-load pass otherwise alternates exp_and_others/natural_log,
# inserting a ~2.7us table load per switch. Entry positions are preserved so
# act_func_set_id indices stay valid.
_orig_gat = bacc.get_activation_tables


def _gat_combined(arch):
    out = {}
    for name, fns in _orig_gat(arch).items():
        if name != "natural_log_exp_and_others":
            fns = {f for f in fns
                   if str(f).split(".")[-1] not in ("Exp", "Ln")}
        out[name] = fns
    return out


bacc.get_activation_tables = _gat_combined

F32 = mybir.dt.float32
F32R = mybir.dt.float32r
F16 = mybir.dt.float16
AF = mybir.ActivationFunctionType
OP = mybir.AluOpType

B, T, D, H, HD = 2, 2048, 1024, 16, 64
HL = 4            # heads per core
OL = HL * HD      # 256 local qkv width
NT = T // 128     # 16 t-tiles
KD = D // 128     # 8 contraction tiles
NQC = T // 512    # 4 q chunks
EPS = 1.1920929e-7
ATTN_SCALE = 0.1
ROPE_BASE = 10000.0
N_CORES = 8

_CACHE = {}


def r32(ap):
    return ap.bitcast(F32R)


def _ptile(pool, shape, dtype, name):
    return pool.tile(shape, dtype, name=name, tag=name, bufs=1)


def build_program():
    nc = bacc.Bacc("TRN2", target_bir_lowering=False, debug=False, num_devices=1)

    xT = nc.dram_tensor("xT", [D, T], F16, kind="ExternalInput").ap()
    xg = nc.dram_tensor("xg", [12, T], F32, kind="ExternalInput").ap()
    wq = nc.dram_tensor("wq", [D, OL], F16, kind="ExternalInput").ap()
    wk = nc.dram_tensor("wk", [D, OL], F16, kind="ExternalInput").ap()
    wv = nc.dram_tensor("wv", [D, OL], F16, kind="ExternalInput").ap()
    v1l = nc.dram_tensor("v1l", [T, OL], F16, kind="ExternalInput").ap()
    wp = nc.dram_tensor("wp", [OL, D], F32, kind="ExternalInput").ap()
    wg = nc.dram_tensor("wg", [12, HL], F32, kind="ExternalInput").ap()
    cos4 = nc.dram_tensor("cos4", [T, 256], F16, kind="ExternalInput").ap()
    sin4 = nc.dram_tensor("sin4", [T, 256], F16, kind="ExternalInput").ap()
    idm = nc.dram_tensor("idm", [128, 256], F32, kind="ExternalInput").ap()
    onec = nc.dram_tensor("onec", [128, NT, HL, 2], F16, kind="ExternalInput").ap()
    msk16 = nc.dram_tensor("msk16", [128, 128], F16, kind="ExternalInput").ap()
    outT = nc.dram_tensor("outT", [D, T], F32, kind="ExternalOutput").ap()

    xT_r = xT.rearrange("(a p) t -> p a t", p=128)   # [128, 8, 2048]
    wq_r = wq.rearrange("(a p) o -> p a o", p=128)   # [128, 8, 256]
    wk_r = wk.rearrange("(a p) o -> p a o", p=128)
    wv_r = wv.rearrange("(a p) o -> p a o", p=128)
    wp_r = wp.rearrange("(a p) o -> p a o", p=128)   # [128, 2, 1024]
    v1_r = v1l.rearrange("(n p) (h d) -> p n h d", p=128, h=HL)
    cs_r = cos4.rearrange("(n p) f -> p n f", p=128)
    sn_r = sin4.rearrange("(n p) f -> p n f", p=128)
    LN01 = float(math.log(ATTN_SCALE))

    with tile.TileContext(nc) as tc:
        import contextlib
        stack = contextlib.ExitStack()
        pers = stack.enter_context(tc.tile_pool(name="pers", bufs=1))
        sp2 = stack.enter_context(tc.tile_pool(name="sp2", bufs=4))
        op_ = stack.enter_context(tc.tile_pool(name="op", bufs=6))
        pp = stack.enter_context(tc.tile_pool(name="pp", bufs=22))
        pss = stack.enter_context(tc.tile_pool(name="pss", bufs=2, space="PSUM"))
        ph1 = contextlib.ExitStack()
        xp = ph1.enter_context(tc.tile_pool(name="xp", bufs=4))
        sp = ph1.enter_context(tc.tile_pool(name="sp", bufs=4))
        psq = ph1.enter_context(tc.tile_pool(name="psq", bufs=3, space="PSUM"))

        # ---- persistent tiles --------------------------------------------
        wp_sb = _ptile(pers, [128, 2, D], F32R, name="wp_sb")
        wg_sb = _ptile(pers, [12, HL], F32R, name="wg_sb")
        idm_sb = _ptile(pers, [128, 256], F32, name="idm_sb")
        msk_sb = _ptile(pers, [128, 128], F16, name="msk_sb")
        id16_sb = _ptile(pers, [128, 128], F16, name="id16_sb")
        eps_sb = _ptile(pers, [128, 1], F32, name="eps_sb")
        nc.vector.memset(eps_sb, EPS)
        ln01_sb = _ptile(pers, [128, 1], F32, name="ln01_sb")
        nc.vector.memset(ln01_sb, LN01)
        # phase-1 weights (needed first — issue their DMAs before anything else)
        wq_sb = _ptile(xp, [128, KD, OL], F16, name="wq_sb")
        wk_sb = _ptile(xp, [128, KD, OL], F16, name="wk_sb")
        wv_sb = _ptile(xp, [128, KD, OL], F16, name="wv_sb")
        nc.gpsimd.dma_start(out=wq_sb, in_=wq_r)
        nc.gpsimd.dma_start(out=wk_sb, in_=wk_r)
        nc.gpsimd.dma_start(out=wv_sb, in_=wv_r)
        nc.gpsimd.dma_start(out=wp_sb, in_=r32(wp_r))
        nc.gpsimd.dma_start(out=wg_sb, in_=r32(wg))
        nc.gpsimd.dma_start(out=idm_sb, in_=idm)
        nc.gpsimd.dma_start(out=msk_sb, in_=msk16)
        nc.vector.tensor_copy(id16_sb, idm_sb[:, 0:128])

        v_m = [_ptile(pers, [128, HL, 66], F16, name=f"v{m}") for m in range(NT)]
        rsk_m = [_ptile(pers, [128, HL], F32, name=f"rsk{m}") for m in range(NT)]
        gate_m = [_ptile(pers, [128, HL], F32, name=f"gate{m}") for m in range(NT)]
        kTt = [[_ptile(pers, [128, 128], F16, name=f"kT{p}_{i}") for i in range(NT)]
               for p in range(2)]
        qTc = [[_ptile(pers, [128, 512], F16, name=f"qT{p}_{c}") for c in range(NQC)]
               for p in range(2)]
        ysb_j = [[_ptile(pers, [128, 128], F32, name=f"ysb{j}_{pp_}")
                  for pp_ in range(2)] for j in range(NT)]
        yTc = [[_ptile(pers, [128, 512], F32R, name=f"yT{a}_{c}") for c in range(NQC)]
               for a in range(2)]

        # ---- phase 1: QKV + gate + v-mix + RMS + RoPE + transpose --------
        for m in range(NT):
            tsl = slice(m * 128, (m + 1) * 128)
            xcol = xp.tile([128, KD, 128], F16, name=f"xcol{m}", tag="xcol")
            nc.sync.dma_start(out=xcol, in_=xT_r[:, :, tsl])
            v1t = sp.tile([128, HL, HD], F16, name=f"v1t{m}", tag="v1t")
            nc.sync.dma_start(out=v1t, in_=v1_r[:, m])
            nc.sync.dma_start(out=v_m[m][:, :, 64:66], in_=onec[:, m])
            ct = sp.tile([128, 256], F16, name=f"ct{m}", tag="ct")
            st = sp.tile([128, 256], F16, name=f"st{m}", tag="st")
            nc.sync.dma_start(out=ct, in_=cs_r[:, m])
            nc.sync.dma_start(out=st, in_=sn_r[:, m])

            q_ps = psq.tile([128, OL], F32, name=f"q_ps{m}", tag="psq")
            k_ps = psq.tile([128, OL], F32, name=f"k_ps{m}", tag="psq")
            v_ps = psq.tile([128, OL], F32, name=f"v_ps{m}", tag="psq")
            for kd in range(KD):
                fl = dict(start=(kd == 0), stop=(kd == KD - 1))
                nc.tensor.matmul(q_ps, xcol[:, kd, :], wq_sb[:, kd, :], **fl)
                nc.tensor.matmul(k_ps, xcol[:, kd, :], wk_sb[:, kd, :], **fl)
                nc.tensor.matmul(v_ps, xcol[:, kd, :], wv_sb[:, kd, :], **fl)

            nc.vector.tensor_add(v_m[m][:, :, 0:64],
                                 v_ps.rearrange("t (h d) -> t h d", h=HL), v1t)

            qk = sp.tile([128, 2, HL, HD], F16, name=f"qk{m}", tag="qk")
            nc.scalar.copy(qk[:, 0], q_ps.rearrange("t (h d) -> t h d", h=HL))
            nc.scalar.copy(qk[:, 1], k_ps.rearrange("t (h d) -> t h d", h=HL))
            qkf = qk.rearrange("t a h d -> t (a h d)")
            scr = sp.tile([128, 2 * OL], F16, name=f"scr{m}", tag="scr")
            ss = sp.tile([128, 2, HL], F32, name=f"ss{m}", tag="ss")
            nc.vector.tensor_mul(scr, qkf, qkf)
            nc.vector.tensor_reduce(
                out=ss.rearrange("t a b -> t (a b)"),
                in_=scr.rearrange("t (g d) -> t g d", d=HD),
                axis=mybir.AxisListType.X, op=OP.add)
            lnv = sp.tile([128, 2, HL], F32, name=f"lnv{m}", tag="lnv")
            nc.scalar.activation(lnv.rearrange("t a b -> t (a b)"),
                                 ss.rearrange("t a b -> t (a b)"),
                                 AF.Ln, scale=1.0 / HD, bias=eps_sb)
            rsq = sp.tile([128, HL], F32, name=f"rsq{m}", tag="rsq")
            nc.scalar.activation(rsq, lnv[:, 0, :], AF.Exp, scale=-0.5)
            nc.scalar.activation(rsk_m[m], lnv[:, 1, :], AF.Exp, scale=-0.5,
                                 bias=ln01_sb)
            for h in range(HL):
                nc.vector.tensor_scalar_mul(qk[:, 0, h, :], qk[:, 0, h, :],
                                             rsq[:, h : h + 1])
                nc.vector.tensor_scalar_mul(qk[:, 1, h, :], qk[:, 1, h, :],
                                             rsk_m[m][:, h : h + 1])

            ctv = ct.rearrange("t (g d) -> t g d", d=32)   # [128, 8, 32]
            stv = st.rearrange("t (g d) -> t g d", d=32)
            qkv_ = qk.rearrange("t a h d -> t (a h) d")    # [128, 8, 64]
            rot = sp.tile([128, 8, HD], F16, name=f"rot{m}", tag="rot")
            tmp = sp.tile([128, 8, 32], F16, name=f"tmp{m}", tag="tmp")
            a, b2 = qkv_[:, :, 0:32], qkv_[:, :, 32:64]
            r1, r2 = rot[:, :, 0:32], rot[:, :, 32:64]
            nc.vector.tensor_mul(tmp, b2, stv)       # x2*sin
            nc.vector.tensor_mul(r1, a, ctv)         # x1*cos
            nc.vector.tensor_add(r1, r1, tmp)
            nc.vector.tensor_mul(tmp, a, stv)        # x1*sin
            nc.vector.tensor_mul(r2, b2, ctv)        # x2*cos
            nc.vector.tensor_sub(r2, r2, tmp)
            rotf = rot.rearrange("t h d -> t (h d)")  # [128, 512] q:0-255 k:256-511
            for half in range(2):
                csl = slice(half * 128, half * 128 + 128)
                tps = pss.tile([128, 128], F16, name=f"tpq{m}{half}", tag="pss")
                nc.tensor.transpose(tps, rotf[:, csl], id16_sb)
                nc.scalar.copy(
                    qTc[half][m // 4][:, 128 * (m % 4) : 128 * (m % 4) + 128],
                    tps)
            for half in range(2):
                csl = slice(256 + half * 128, 256 + half * 128 + 128)
                tps = pss.tile([128, 128], F16, name=f"tpk{m}{half}", tag="pss")
                nc.tensor.transpose(tps, rotf[:, csl], id16_sb)
                nc.vector.tensor_copy(kTt[half][m], tps)

        ph1.close()
        psy = stack.enter_context(tc.tile_pool(name="psy", bufs=2, space="PSUM"))
        prp = stack.enter_context(tc.tile_pool(name="prp", bufs=2, space="PSUM"))

        # ---- gate: sigmoid(x[:, :12] @ Wg.T) computed from xg strip ------
        xg_sb = _ptile(pers, [12, T], F32R, name="xg_sb")
        nc.sync.dma_start(out=xg_sb, in_=r32(xg))
        for m in range(NT):
            tsl = slice(m * 128, (m + 1) * 128)
            g_ps = prp.tile([128, HL], F32, name=f"g_ps{m}", tag="prp")
            nc.tensor.matmul(g_ps, xg_sb[:, tsl], wg_sb, start=True, stop=True)
            gtmp = sp2.tile([128, HL], F32, name=f"gtmp{m}", tag="gtmp")
            nc.scalar.activation(gtmp, g_ps, AF.Exp, scale=-1.0)
            nc.vector.tensor_scalar_add(gtmp, gtmp, 1.0)
            nc.vector.reciprocal(gate_m[m], gtmp)

        # ---- phase 2: attention (qc outer), y transpose + c_proj per qc --
        for qc in range(NQC):
            for p in range(2):
                pts2 = []
                for i in range(4 * qc + 4):
                    ql0 = max(0, 128 * (i - 4 * qc))
                    ncols = 512 - ql0
                    s_ps = pss.tile([128, 2, 512], F32, name=f"s{p}_{qc}_{i}",
                                    tag="pss")
                    for r in range(2):
                        prt = slice(64 * r, 64 * r + 64)
                        nc.tensor.matmul(
                            s_ps[:, r, 0:ncols],
                            kTt[p][i][prt, :],
                            qTc[p][qc][prt, ql0:512],
                            start=True, stop=True,
                            tile_position=(64 * r, 0))
                    pt_t = pp.tile([128, 2, 512], F16, name=f"pt{p}_{qc}_{i}",
                                   tag="pt")
                    nc.scalar.activation(pt_t[:, :, ql0:512], s_ps[:, :, 0:ncols],
                                         AF.Exp)
                    if i >= 4 * qc:
                        for r in range(2):
                            nc.vector.tensor_mul(pt_t[:, r, ql0 : ql0 + 128],
                                                 pt_t[:, r, ql0 : ql0 + 128],
                                                 msk_sb)
                    pts2.append(pt_t)
                for r in range(2):
                    h = 2 * p + r
                    for js in range(4):
                        j = 4 * qc + js
                        y_ps = psy.tile([128, 66], F32, name=f"y{h}_{j}", tag="psy")
                        for i in range(j + 1):
                            nc.tensor.matmul(
                                y_ps,
                                pts2[i][:, r, 128 * js : 128 * js + 128],
                                v_m[i][:, h, 0:66],
                                start=(i == 0), stop=(i == j))
                        rec = sp2.tile([128, 1], F32, name=f"rec{h}_{j}", tag="rec")
                        nc.vector.reciprocal(rec, y_ps[:, 64:65])
                        nc.vector.tensor_scalar(
                            out=ysb_j[j][p][:, HD * r : HD * r + HD],
                            in0=y_ps[:, 0:64], scalar1=rec,
                            scalar2=gate_m[j][:, h : h + 1],
                            op0=OP.mult, op1=OP.mult)

            # y transpose for this qc's 4 j-tiles, then c_proj chunk
            for js in range(4):
                j = 4 * qc + js
                for half in range(2):
                    tps = prp.tile([128, 128], F32, name=f"ty{j}{half}",
                                   tag="prp")
                    nc.tensor.transpose(tps, ysb_j[j][half], idm_sb[:, 0:128])
                    nc.vector.tensor_copy(
                        yTc[half][qc][:, 128 * js : 128 * js + 128], tps)
            tsl = slice(512 * qc, 512 * qc + 512)
            for oc in range(8):
                osl = slice(128 * oc, 128 * oc + 128)
                pr_ps = prp.tile([128, 512], F32, name=f"pr{oc}_{qc}", tag="prp")
                for a in range(2):
                    nc.tensor.matmul(pr_ps, r32(wp_sb[:, a, osl]), yTc[a][qc],
                                     start=(a == 0), stop=(a == 1))
                ot = op_.tile([128, 512], F32, name=f"ot{oc}_{qc}", tag="ot")
                nc.vector.tensor_copy(ot, pr_ps)
                nc.sync.dma_start(out=outT[osl, tsl], in_=ot)

        stack.close()

    nc.compile()
    return nc


def _host_prep(x, v1, Wq, Wk, Wv, Wproj, Wg, lamb):
    lamb = np.float32(lamb)
    half = HD // 2
    inv_freq = 1.0 / (ROPE_BASE ** (np.arange(0, HD, 2, dtype=np.float32) / HD))
    freqs = np.arange(T, dtype=np.float32)[:, None] * inv_freq[None, :]  # [T, 32]
    cos4 = np.ascontiguousarray(np.tile(np.cos(freqs), (1, 2 * HL))).astype(np.float16)
    sin4 = np.ascontiguousarray(np.tile(np.sin(freqs), (1, 2 * HL))).astype(np.float16)

    idm = np.zeros((128, 256), dtype=np.float32)
    idm[:, 0:128] = np.eye(128, dtype=np.float32)
    # causal keep-mask in S^T layout: rows=k_local, cols=q_local; keep q >= k
    kk, qq = np.meshgrid(np.arange(128), np.arange(128), indexing="ij")
    idm[:, 128:256] = (qq >= kk).astype(np.float32)

    WqT = np.ascontiguousarray(Wq.T)  # [D, D]; col o = head o//64
    WkT = np.ascontiguousarray(Wk.T)
    WvT = np.ascontiguousarray(Wv.T) * (1.0 - lamb)

    in_maps = []
    for c in range(N_CORES):
        b, g = c // 4, c % 4
        hsl = slice(g * OL, (g + 1) * OL)
        in_maps.append({
            "xT": np.ascontiguousarray(x[b].T).astype(np.float16),
            "xg": np.ascontiguousarray(x[b, :, :12].T),
            "wq": np.ascontiguousarray(WqT[:, hsl]).astype(np.float16),
            "wk": np.ascontiguousarray(WkT[:, hsl]).astype(np.float16),
            "wv": np.ascontiguousarray(WvT[:, hsl]).astype(np.float16),
            "v1l": np.ascontiguousarray(
                v1[b, :, g * HL : (g + 1) * HL, :].reshape(T, OL)
                * lamb).astype(np.float16),
            "wp": np.ascontiguousarray(Wproj[:, hsl].T),
            "wg": np.ascontiguousarray(Wg[g * HL : (g + 1) * HL, :].T),
            "cos4": cos4,
            "sin4": sin4,
            "idm": idm,
            "onec": np.ones((128, NT, HL, 2), dtype=np.float16),
            "msk16": idm[:, 128:256].astype(np.float16),
        })
    return in_maps


def kernel(x, v1, Wq, Wk, Wv, Wproj, Wg, lamb, **run_kwargs):
    x = np.asarray(x, dtype=np.float32)
    v1 = np.asarray(v1, dtype=np.float32)
    if "nc" not in _CACHE:
        _CACHE["nc"] = build_program()
    nc = _CACHE["nc"]
    in_maps = _host_prep(x, np.asarray(v1), np.asarray(Wq), np.asarray(Wk),
                         np.asarray(Wv), np.asarray(Wproj), np.asarray(Wg),
                         np.float32(lamb))
    res = run_bass_kernel_spmd(nc, in_maps, core_ids=list(range(N_CORES)),
                               **run_kwargs)
    _CACHE["last_results"] = res
    out = np.zeros((B, T, D), dtype=np.float32)
    for c in range(N_CORES):
        b = c // 4
        out[b] += res.results[c]["outT"].T
    return out

